# revision 16
# baseline (speedup 1.0000x reference)
"""Trainium2 Bass kernel for topk_masking:  out = X + alpha * (top32_mask(A) @ X).

Row-parallel across 8 NeuronCores (A sharded [1024, 8192] per core, X
replicated).  Per 128-row batch on each core, split into L/R half-rows for
pipelining:
  * VectorE: per-segment max8 over each half -> candidate top-8s, then 4
    rounds of max+match_replace over the candidates -> top-32 values;
    t32 = 32nd largest.  Exact unless >8 of a row's top-32 fall in one
    segment (9 rows for this data; detected and host-fixed).
  * ScalarE: maskpm = Sign(A - prevfloat(t32)) in bf16 (+1 selected, -1 not),
    with fused accumulation as an exactness detector (catches segment
    overflow, boundary-value ties, Sign==0).
  * GPSIMD dma_gather(transpose): maskpm half -> maskT chunks [128j, 128row].
  * TensorE: psum = maskpm @ Xb (64 accumulated chunk matmuls, bf16).
    mask01 @ X = (maskpm @ X + colsum(X)) / 2, so
    out = [X_self + (a/2) colsum] + (a/2) psum, with colsum(Xb) from an
    all-ones matmul once per core.
  * VectorE: out = (a/2) * psum + Xmod; DMA out.
Host: rows whose detector count != 32 are recomputed exactly (~11 rows).
"""

import os
import numpy as np

N = 8192
HALF = N // 2
D = 256
K = 32
NCORES = 8
RPC = N // NCORES          # rows per core = 1024
BATCH = 128
NBATCH = RPC // BATCH      # 8
SEG = int(os.environ.get("TOPK_SEG", "512"))
NCH = N // 128             # 64 contraction chunks
HCH = NCH // 2
NEG_BIG = -1e30
ONE_MINUS_EPS = float(np.float32(1.0) - np.float32(2.0 ** -24))

last_results = None


def _build(loop_reps=1, seg=None):
    import concourse.bacc as bacc
    import concourse.mybir as mybir
    from concourse.tile import TileContext
    from concourse import library_config

    seg = seg or SEG
    nseg = N // seg            # segments per full row
    hseg = nseg // 2           # segments per half
    fp32 = mybir.dt.float32
    bf16 = mybir.dt.bfloat16
    add = mybir.AluOpType.add
    mult = mybir.AluOpType.mult
    Sign = mybir.ActivationFunctionType.Sign
    Copy = mybir.ActivationFunctionType.Copy

    nc = bacc.Bacc("TRN2", debug=False)
    a_in = nc.declare_dram_parameter("a", [RPC, N], fp32, isOutput=False)
    xb_in = nc.declare_dram_parameter("xb", [N, D], bf16, isOutput=False)
    xs_in = nc.declare_dram_parameter("xself", [RPC, D], fp32, isOutput=False)
    al_in = nc.declare_dram_parameter("alpha_h", [128, 1], fp32, isOutput=False)
    ti_in = nc.declare_dram_parameter("tidx", [128, 8], mybir.dt.int16, isOutput=False)
    out_ext = nc.declare_dram_parameter("out", [RPC, D], fp32, isOutput=True)
    cnt_ext = nc.declare_dram_parameter("count", [RPC, 1], fp32, isOutput=True)

    abufs = int(os.environ.get("TOPK_ABUFS", "3"))

    with TileContext(nc) as tc:
        with (
            tc.tile_pool(name="persist", bufs=1) as persist,
            tc.tile_pool(name="apool", bufs=abufs) as apool,
            tc.tile_pool(name="mpool", bufs=2) as mpool,
            tc.tile_pool(name="mtpool", bufs=2) as mtpool,
            tc.tile_pool(name="small", bufs=2) as small,
            tc.tile_pool(name="psum", bufs=2, space="PSUM") as psum_pool,
            tc.tile_pool(name="psumc", bufs=1, space="PSUM") as psumc_pool,
        ):
            nc.gpsimd.load_library(library_config.mlp)

            at_tiles = {}

            def load_at(b):
                atL = apool.tile([128, HALF], fp32, tag="atL")
                atR = apool.tile([128, HALF], fp32, tag="atR")
                nc.sync.dma_start(out=atL[:], in_=a_in[b * BATCH:(b + 1) * BATCH, 0:HALF])
                nc.sync.dma_start(out=atR[:], in_=a_in[b * BATCH:(b + 1) * BATCH, HALF:N])
                at_tiles[b] = (atL, atR)

            if loop_reps == 1:
                load_at(0)
                load_at(1)

            tidx = persist.tile([128, 8], mybir.dt.int16)
            nc.scalar.dma_start(out=tidx[:], in_=ti_in[:])

            # X resident in bf16, chunk-major: xb[p, c*D + d] = X[c*128 + p, d]
            xb = persist.tile([128, NCH * D], bf16)
            nc.scalar.dma_start(
                out=xb[:].rearrange("p (c d) -> p c d", d=D),
                in_=xb_in.rearrange("(c p) d -> p c d", p=128),
            )
            alpha_h = persist.tile([128, 1], fp32)
            nc.scalar.dma_start(out=alpha_h[:], in_=al_in[:])
            cnt_all = persist.tile([128, NBATCH], fp32)

            xv = xb[:].rearrange("p (c d) -> p c d", d=D)

            # colsum(Xb) broadcast to 128 rows via all-ones matmul
            ones_sb = persist.tile([128, 128], bf16)
            nc.vector.memset(ones_sb[:], 1.0)
            ps_cs = psumc_pool.tile([128, D], fp32)
            for c in range(NCH):
                nc.tensor.matmul(ps_cs[:], lhsT=ones_sb[:], rhs=xv[:, c, :],
                                 start=(c == 0), stop=(c == NCH - 1))

            # Xmod = X_self + (alpha/2) * colsum   (per 128-row slice)
            xmod = persist.tile([128, NBATCH * D], fp32)
            for b in range(NBATCH):
                xs = small.tile([128, D], fp32)
                nc.scalar.dma_start(out=xs[:], in_=xs_in[b * BATCH:(b + 1) * BATCH, :])
                nc.vector.scalar_tensor_tensor(
                    out=xmod[:, b * D:(b + 1) * D], in0=ps_cs[:],
                    scalar=alpha_h[:, 0:1], in1=xs[:], op0=mult, op1=add)

            def batch_body(b):
                if b + 2 < NBATCH:
                    load_at(b + 2)
                atL, atR = at_tiles.pop(b)

                # per-segment top-8 candidates (L half then R half)
                cands = small.tile([128, nseg * 8], fp32)
                for s in range(hseg):
                    nc.vector.max(out=cands[:, s * 8:(s + 1) * 8],
                                  in_=atL[:, s * seg:(s + 1) * seg])
                for s in range(hseg):
                    nc.vector.max(out=cands[:, (hseg + s) * 8:(hseg + s + 1) * 8],
                                  in_=atR[:, s * seg:(s + 1) * seg])

                # top-32 of candidates -> t32
                v8 = small.tile([128, K], fp32)
                for r in range(4):
                    nc.vector.max(out=v8[:, r * 8:(r + 1) * 8], in_=cands[:])
                    if r < 3:
                        nc.vector.match_replace(
                            out=cands[:], in_to_replace=v8[:, r * 8:(r + 1) * 8],
                            in_values=cands[:], imm_value=NEG_BIG)

                # neg_tprime = -prevfloat(t32) = t32 * -(1 - 2^-24)   (on ACT)
                ntp = small.tile([128, 1], fp32)
                nc.scalar.activation(out=ntp[:], in_=v8[:, K - 1:K], func=Copy,
                                     scale=-ONE_MINUS_EPS)

                # maskpm = Sign(A - prevfloat(t32)) in {+1,-1} bf16, halves;
                # accum halves summed -> detector (== 2K - N iff exact)
                ps = psum_pool.tile([128, D], fp32)
                acc2 = small.tile([128, 2], fp32)
                for h, at_h in ((0, atL), (1, atR)):
                    maskb = mpool.tile([128, HALF], bf16, tag=f"mb{h}")
                    nc.scalar.activation(
                        out=maskb[:], in_=at_h[:], func=Sign,
                        bias=ntp[:, 0:1], scale=1.0,
                        accum_out=acc2[:, h:h + 1])

                    # transpose half: maskT[p, c, i] = maskpm_h[i, c*128+p]
                    maskT = mtpool.tile([128, HCH * 128], bf16, tag=f"mt{h}")
                    nc.gpsimd.dma_gather(
                        out_ap=maskT[:].rearrange("p (c i) -> p c i", i=128),
                        in_ap=maskb[:], idxs_ap=tidx[:],
                        num_idxs=128, num_idxs_reg=128, elem_size=HALF,
                        transpose=True,
                        sbuf_tokens_per_rank=128, sbuf_free_dim_per_rank=HALF * 2)
                    mT = maskT[:].rearrange("p (c i) -> p c i", i=128)

                    for c in range(HCH):
                        nc.tensor.matmul(
                            ps[:], lhsT=mT[:, c, :], rhs=xv[:, h * HCH + c, :],
                            start=(h == 0 and c == 0),
                            stop=(h == 1 and c == HCH - 1))

                nc.vector.tensor_add(out=cnt_all[:, b:b + 1], in0=acc2[:, 0:1],
                                     in1=acc2[:, 1:2])

                # out = (alpha/2) * psum + Xmod
                ot = small.tile([128, D], fp32)
                nc.vector.scalar_tensor_tensor(
                    out=ot[:], in0=ps[:], scalar=alpha_h[:, 0:1],
                    in1=xmod[:, b * D:(b + 1) * D], op0=mult, op1=add)
                nc.sync.dma_start(out=out_ext[b * BATCH:(b + 1) * BATCH, :], in_=ot[:])

            if loop_reps == 1:
                for b in range(NBATCH):
                    batch_body(b)
            else:
                with tc.For_i(0, loop_reps, 1):
                    load_at(0)
                    load_at(1)
                    for b in range(NBATCH):
                        batch_body(b)

            # counts: cnt_all[p, b] -> count[b*128 + p]
            nc.sync.dma_start(
                out=cnt_ext.rearrange("(b p) one -> p (b one)", p=128),
                in_=cnt_all[:],
            )
    nc.compile()
    return nc


def _tidx():
    t = np.zeros((16, 8), np.int16)
    for i in range(128):
        t[i % 16, i // 16] = i
    return np.tile(t, (8, 1))


def make_in_maps(A, X, alpha):
    import ml_dtypes
    Xb = X.astype(ml_dtypes.bfloat16)
    alpha_h = np.full((128, 1), np.float32(alpha) / np.float32(2.0), np.float32)
    tidx = _tidx()
    return [{
        "a": A[c * RPC:(c + 1) * RPC],
        "xb": Xb,
        "xself": X[c * RPC:(c + 1) * RPC],
        "alpha_h": alpha_h,
        "tidx": tidx,
    } for c in range(NCORES)]


def kernel(**inputs):
    global last_results
    from concourse.bass_utils import run_bass_kernel_spmd

    A = np.ascontiguousarray(np.asarray(inputs["A"], dtype=np.float32))
    X = np.ascontiguousarray(np.asarray(inputs["X"], dtype=np.float32))
    alpha = np.float32(np.asarray(inputs["alpha"]))
    k = int(np.asarray(inputs["k"]))
    assert A.shape == (N, N) and X.shape == (N, D) and k == K

    nc = _build()
    in_maps = make_in_maps(A, X, alpha)

    trace = bool(int(os.environ.get("TOPK_TRACE", "0")))
    res = run_bass_kernel_spmd(nc, in_maps, core_ids=list(range(NCORES)),
                               trace=trace)
    last_results = res

    out = np.concatenate([r["out"] for r in res.results], axis=0)
    accs = np.concatenate([r["count"] for r in res.results], axis=0)[:, 0]

    # Host fallback for rows where the device selection is not exactly top-k
    # (boundary value ties, segment overflow, Sign hitting exact zero).
    bad = np.flatnonzero(accs != np.float32(2 * K - N))
    for r in bad:
        order = np.argsort(-A[r], kind="stable")[:K]
        out[r] = X[r] + alpha * X[order].sum(axis=0, dtype=np.float32)

    return out.astype(np.float32, copy=False)


# revision 20
# speedup vs baseline: 7.0362x; 7.0362x over previous
"""Trainium2 Bass kernel for topk_masking:  out = X + alpha * (top32_mask(A) @ X).

Row-parallel across 8 NeuronCores (A sharded [1024, 8192] per core, X
replicated).  Per 128-row batch on each core, split into L/R half-rows for
pipelining:
  * VectorE: per-segment max8 over each half -> candidate top-8s, then 4
    rounds of max+match_replace over the candidates -> top-32 values;
    t32 = 32nd largest.  Exact unless >8 of a row's top-32 fall in one
    segment (9 rows for this data; detected and host-fixed).
  * ScalarE: maskpm = Sign(A - prevfloat(t32)) in bf16 (+1 selected, -1 not),
    with fused accumulation as an exactness detector (catches segment
    overflow, boundary-value ties, Sign==0).
  * GPSIMD dma_gather(transpose): maskpm half -> maskT chunks [128j, 128row].
  * TensorE: psum = maskpm @ Xb (64 accumulated chunk matmuls, bf16).
    mask01 @ X = (maskpm @ X + colsum(X)) / 2, so
    out = [X_self + (a/2) colsum] + (a/2) psum, with colsum(Xb) from an
    all-ones matmul once per core.
  * VectorE: out = (a/2) * psum + Xmod; DMA out.
Host: rows whose detector count != 32 are recomputed exactly (~11 rows).
"""

import os
import numpy as np

N = 8192
HALF = N // 2
D = 256
K = 32
NCORES = 8
RPC = N // NCORES          # rows per core = 1024
BATCH = 128
NBATCH = RPC // BATCH      # 8
SEG = int(os.environ.get("TOPK_SEG", "512"))
NCH = N // 128             # 64 contraction chunks
HCH = NCH // 2
NEG_BIG = -1e30
ONE_MINUS_EPS = float(np.float32(1.0) - np.float32(2.0 ** -24))

last_results = None
_nc_cache = {}


def _build_cached(loop_reps=1, seg=None):
    key = (loop_reps, seg or SEG)
    if key not in _nc_cache:
        _nc_cache[key] = _build(loop_reps, seg)
    return _nc_cache[key]


def _build(loop_reps=1, seg=None):
    import concourse.bacc as bacc
    import concourse.mybir as mybir
    from concourse.tile import TileContext
    from concourse import library_config

    seg = seg or SEG
    nseg = N // seg            # segments per full row
    hseg = nseg // 2           # segments per half
    fp32 = mybir.dt.float32
    bf16 = mybir.dt.bfloat16
    add = mybir.AluOpType.add
    mult = mybir.AluOpType.mult
    Sign = mybir.ActivationFunctionType.Sign
    Copy = mybir.ActivationFunctionType.Copy

    nc = bacc.Bacc("TRN2", debug=False, num_swdge_queues=2)
    a_in = nc.declare_dram_parameter("a", [RPC, N], fp32, isOutput=False)
    xb_in = nc.declare_dram_parameter("xb", [N, D], bf16, isOutput=False)
    xs_in = nc.declare_dram_parameter("xself", [RPC, D], fp32, isOutput=False)
    al_in = nc.declare_dram_parameter("alpha_h", [128, 1], fp32, isOutput=False)
    ti_in = nc.declare_dram_parameter("tidx", [128, 8], mybir.dt.int16, isOutput=False)
    out_ext = nc.declare_dram_parameter("out", [RPC, D], fp32, isOutput=True)
    cnt_ext = nc.declare_dram_parameter("count", [RPC, 1], fp32, isOutput=True)

    abufs = int(os.environ.get("TOPK_ABUFS", "3"))

    with TileContext(nc) as tc:
        with (
            tc.tile_pool(name="persist", bufs=1) as persist,
            tc.tile_pool(name="apool", bufs=abufs) as apool,
            tc.tile_pool(name="mpool", bufs=2) as mpool,
            tc.tile_pool(name="mtpool", bufs=2) as mtpool,
            tc.tile_pool(name="small", bufs=2) as small,
            tc.tile_pool(name="psum", bufs=2, space="PSUM") as psum_pool,
            tc.tile_pool(name="psumc", bufs=1, space="PSUM") as psumc_pool,
        ):
            nc.gpsimd.load_library(library_config.mlp)

            at_tiles = {}

            def load_at(b):
                atL = apool.tile([128, HALF], fp32, tag="atL")
                atR = apool.tile([128, HALF], fp32, tag="atR")
                nc.sync.dma_start(out=atL[:], in_=a_in[b * BATCH:(b + 1) * BATCH, 0:HALF])
                nc.sync.dma_start(out=atR[:], in_=a_in[b * BATCH:(b + 1) * BATCH, HALF:N])
                at_tiles[b] = (atL, atR)

            if loop_reps == 1:
                load_at(0)
                load_at(1)

            tidx = persist.tile([128, 8], mybir.dt.int16)
            nc.scalar.dma_start(out=tidx[:], in_=ti_in[:])

            # X resident in bf16, chunk-major: xb[p, c*D + d] = X[c*128 + p, d]
            xb = persist.tile([128, NCH * D], bf16)
            nc.scalar.dma_start(
                out=xb[:].rearrange("p (c d) -> p c d", d=D),
                in_=xb_in.rearrange("(c p) d -> p c d", p=128),
            )
            alpha_h = persist.tile([128, 1], fp32)
            nc.scalar.dma_start(out=alpha_h[:], in_=al_in[:])
            cnt_all = persist.tile([128, NBATCH], fp32)

            xv = xb[:].rearrange("p (c d) -> p c d", d=D)

            # colsum(Xb) broadcast to 128 rows via all-ones matmul
            ones_sb = persist.tile([128, 128], bf16)
            nc.vector.memset(ones_sb[:], 1.0)
            ps_cs = psumc_pool.tile([128, D], fp32)
            for c in range(NCH):
                nc.tensor.matmul(ps_cs[:], lhsT=ones_sb[:], rhs=xv[:, c, :],
                                 start=(c == 0), stop=(c == NCH - 1))

            # Xmod = X_self + (alpha/2) * colsum   (per 128-row slice)
            xmod = persist.tile([128, NBATCH * D], fp32)
            for b in range(NBATCH):
                xs = small.tile([128, D], fp32)
                nc.scalar.dma_start(out=xs[:], in_=xs_in[b * BATCH:(b + 1) * BATCH, :])
                nc.vector.scalar_tensor_tensor(
                    out=xmod[:, b * D:(b + 1) * D], in0=ps_cs[:],
                    scalar=alpha_h[:, 0:1], in1=xs[:], op0=mult, op1=add)

            def batch_body(b):
                if b + 2 < NBATCH:
                    load_at(b + 2)
                atL, atR = at_tiles.pop(b)

                # per-segment top-8 candidates (L half then R half)
                cands = small.tile([128, nseg * 8], fp32)
                for s in range(hseg):
                    nc.vector.max(out=cands[:, s * 8:(s + 1) * 8],
                                  in_=atL[:, s * seg:(s + 1) * seg])
                for s in range(hseg):
                    nc.vector.max(out=cands[:, (hseg + s) * 8:(hseg + s + 1) * 8],
                                  in_=atR[:, s * seg:(s + 1) * seg])

                # top-32 of candidates -> t32
                v8 = small.tile([128, K], fp32)
                for r in range(4):
                    nc.vector.max(out=v8[:, r * 8:(r + 1) * 8], in_=cands[:])
                    if r < 3:
                        nc.vector.match_replace(
                            out=cands[:], in_to_replace=v8[:, r * 8:(r + 1) * 8],
                            in_values=cands[:], imm_value=NEG_BIG)

                # neg_tprime = -prevfloat(t32) = t32 * -(1 - 2^-24)   (on ACT)
                ntp = small.tile([128, 1], fp32)
                nc.scalar.activation(out=ntp[:], in_=v8[:, K - 1:K], func=Copy,
                                     scale=-ONE_MINUS_EPS)

                # maskpm = Sign(A - prevfloat(t32)) in {+1,-1} bf16, halves;
                # accum halves summed -> detector (== 2K - N iff exact)
                ps = psum_pool.tile([128, D], fp32)
                acc2 = small.tile([128, 2], fp32)
                for h, at_h in ((0, atL), (1, atR)):
                    maskb = mpool.tile([128, HALF], bf16, tag=f"mb{h}")
                    nc.scalar.activation(
                        out=maskb[:], in_=at_h[:], func=Sign,
                        bias=ntp[:, 0:1], scale=1.0,
                        accum_out=acc2[:, h:h + 1])

                    # transpose half: maskT[p, c, i] = maskpm_h[i, c*128+p]
                    maskT = mtpool.tile([128, HCH * 128], bf16, tag=f"mt{h}")
                    nc.gpsimd.dma_gather(
                        out_ap=maskT[:].rearrange("p (c i) -> p c i", i=128),
                        in_ap=maskb[:], idxs_ap=tidx[:],
                        num_idxs=128, num_idxs_reg=128, elem_size=HALF,
                        transpose=True, queue_num=h,
                        sbuf_tokens_per_rank=128, sbuf_free_dim_per_rank=HALF * 2)
                    mT = maskT[:].rearrange("p (c i) -> p c i", i=128)

                    for c in range(HCH):
                        nc.tensor.matmul(
                            ps[:], lhsT=mT[:, c, :], rhs=xv[:, h * HCH + c, :],
                            start=(h == 0 and c == 0),
                            stop=(h == 1 and c == HCH - 1))

                nc.vector.tensor_add(out=cnt_all[:, b:b + 1], in0=acc2[:, 0:1],
                                     in1=acc2[:, 1:2])

                # out = (alpha/2) * psum + Xmod
                ot = small.tile([128, D], fp32)
                nc.vector.scalar_tensor_tensor(
                    out=ot[:], in0=ps[:], scalar=alpha_h[:, 0:1],
                    in1=xmod[:, b * D:(b + 1) * D], op0=mult, op1=add)
                nc.sync.dma_start(out=out_ext[b * BATCH:(b + 1) * BATCH, :], in_=ot[:])

            if loop_reps == 1:
                for b in range(NBATCH):
                    batch_body(b)
            else:
                with tc.For_i(0, loop_reps, 1):
                    load_at(0)
                    load_at(1)
                    for b in range(NBATCH):
                        batch_body(b)

            # counts: cnt_all[p, b] -> count[b*128 + p]
            nc.sync.dma_start(
                out=cnt_ext.rearrange("(b p) one -> p (b one)", p=128),
                in_=cnt_all[:],
            )
    nc.compile()
    return nc


def _tidx():
    t = np.zeros((16, 8), np.int16)
    for i in range(128):
        t[i % 16, i // 16] = i
    return np.tile(t, (8, 1))


def make_in_maps(A, X, alpha):
    import ml_dtypes
    Xb = X.astype(ml_dtypes.bfloat16)
    alpha_h = np.full((128, 1), np.float32(alpha) / np.float32(2.0), np.float32)
    tidx = _tidx()
    return [{
        "a": A[c * RPC:(c + 1) * RPC],
        "xb": Xb,
        "xself": X[c * RPC:(c + 1) * RPC],
        "alpha_h": alpha_h,
        "tidx": tidx,
    } for c in range(NCORES)]


def kernel(**inputs):
    global last_results
    from concourse.bass_utils import run_bass_kernel_spmd

    A = np.ascontiguousarray(np.asarray(inputs["A"], dtype=np.float32))
    X = np.ascontiguousarray(np.asarray(inputs["X"], dtype=np.float32))
    alpha = np.float32(np.asarray(inputs["alpha"]))
    k = int(np.asarray(inputs["k"]))
    assert A.shape == (N, N) and X.shape == (N, D) and k == K

    nc = _build_cached()
    in_maps = make_in_maps(A, X, alpha)

    trace = bool(int(os.environ.get("TOPK_TRACE", "0")))
    res = run_bass_kernel_spmd(nc, in_maps, core_ids=list(range(NCORES)),
                               trace=trace)
    last_results = res

    out = np.concatenate([r["out"] for r in res.results], axis=0)
    accs = np.concatenate([r["count"] for r in res.results], axis=0)[:, 0]

    # Host fallback for rows where the device selection is not exactly top-k
    # (boundary value ties, segment overflow, Sign hitting exact zero).
    bad = np.flatnonzero(accs != np.float32(2 * K - N))
    for r in bad:
        order = np.argsort(-A[r], kind="stable")[:K]
        out[r] = X[r] + alpha * X[order].sum(axis=0, dtype=np.float32)

    return out.astype(np.float32, copy=False)
